# revision 34
# baseline (speedup 1.0000x reference)
"""DigitCaps routing kernel for 8 Trainium2 NeuronCores.

Sharding: input_dim (1024 primary capsules) split 8 ways; per-core
preactivation partial sums are AllReduced each routing iteration.

Per core (I_c = 128 local capsules = 64 pairs):
  phase B: votes_pair = S_p.T @ Wp_p with S_p a host-built block-diagonal
           [128,128] stationary (inputsT of the 2 capsules on the diagonal),
           so one moving stream of W computes both capsules' votes.
           Uniform-route preactivation (iteration 0) accumulates on DVE in
           four interleaved bf16 chains, combined in f32.
           A tiny dummy AllReduce early in phase B absorbs collective
           warmup + core skew so the real AllReduces run at floor latency.
  pass k (k=1,2): stream votes (SBUF-resident for the first R pairs, DRAM
           for the rest), compute agreement delta -> logits -> leaky softmax
           route -> route-weighted partial preactivation (PE identity-
           accumulate).
  Each iteration: AllReduce [64, 2048] partials, squash locally.

All per-(b,o,a) tensors are kept a-major (a outer, o inner) so the
agreement fold over atoms is a contiguous halving chain.
"""

import sys

if '/opt/trn_rl_repo' not in sys.path:
    sys.path.insert(0, '/opt/trn_rl_repo')

import numpy as np
import ml_dtypes

import concourse.bacc as bacc
import concourse.mybir as mybir
import concourse.tile as tile
from concourse import masks
from concourse.bass_utils import run_bass_kernel_spmd

N_CORES = 8
B = 64          # batch
I_FULL = 1024   # primary capsules
C = 64          # input atoms
O = 64          # output capsules
A = 32          # output atoms
OA = O * A      # 2048
IC = I_FULL // N_CORES   # 128 local capsules
NPAIR = IC // 2          # 64
NGB = 4                  # pairs per streaming group
R = 8                    # SBUF-resident pairs (must be multiple of NGB)

f32 = mybir.dt.float32
bf16 = mybir.dt.bfloat16
f8 = mybir.dt.float8e4
bf16_np = ml_dtypes.bfloat16
f8_np = ml_dtypes.float8_e4m3

W_SCALE = 64.0  # host pre-scale so fp8 W stays in normal range

LEAK_SCALE = 1.0 / (O + 1)  # route0 value: softmax of 65 zero logits


def _squash_factors(nc, pre, nsq, nrm, den, rec, fac, sq):
    """pre [P, OA] f32 a-major -> fac [P, O] f32 (act = pre * fac)."""
    nc.vector.tensor_mul(sq, pre, pre)
    nc.vector.reduce_sum(
        nsq, sq.rearrange("p (a o) -> p o a", o=O),
        axis=mybir.AxisListType.X,
    )
    nc.scalar.sqrt(nrm, nsq)
    nc.scalar.add(den, nsq, 1.0)
    nc.vector.reciprocal(rec, den)
    nc.vector.tensor_mul(fac, nrm, rec)


def _build(sim_mode=False):
    nc = bacc.Bacc("TRN2", target_bir_lowering=False, debug=False,
                   num_devices=1 if sim_mode else N_CORES)
    with tile.TileContext(nc) as tc:
        _emit(nc, tc, sim_mode)
    nc.compile()
    return nc


def _emit(nc, tc, sim_mode=False):
    s_d = nc.dram_tensor("S", [128, NPAIR * 128], bf16, kind="ExternalInput")
    # W partition-major: per partition one contiguous (pair, oa) run, so a
    # 4-pair group load is 128 contiguous 16 KB descriptors (near line rate)
    wp_d = nc.dram_tensor("Wp", [128, NPAIR * OA], bf16, kind="ExternalInput")
    b_d = nc.dram_tensor("biases", [OA], f32, kind="ExternalInput")
    out_d = nc.dram_tensor("out", [B, O], f32, kind="ExternalOutput")

    with (
        tc.tile_pool(name="const", bufs=1) as cpool,
        tc.tile_pool(name="persist", bufs=1) as ppool,
        tc.tile_pool(name="dram", bufs=1, space="DRAM") as dpool,
    ):
        ident16 = cpool.tile([128, 128], bf16)
        masks.make_identity(nc, ident16[:])
        # parity-fold stationary: 1 at (k, k%64) -- folds the two capsule
        # parity halves during the PE preactivation accumulate
        fold64 = cpool.tile([128, 64], bf16)
        nc.vector.tensor_add(fold64[:], ident16[:, 0:64], ident16[:, 64:128])
        bias_sb = cpool.tile([128, OA], f32)  # a-major broadcast
        nc.scalar.dma_start(
            bias_sb[:], b_d[:].unsqueeze(0).broadcast_to((128, OA))
        )

        votes_sb = ppool.tile([128, R * OA], bf16)
        logits = ppool.tile([128, NPAIR * O], bf16)  # [(par,b), (pair, o)]
        votes_dram = dpool.tile([(NPAIR - R) // NGB, 128, NGB * OA], bf16)
        ar_in = dpool.tile([B, OA], bf16)
        ar_out = dpool.tile([B, OA], bf16)
        warm_in = dpool.tile([B, OA], bf16)
        warm_out = dpool.tile([B, OA], bf16)

        def _all_reduce(a_in, a_out):
            if sim_mode:
                nc.sync.dma_start(a_out[:], a_in[:])
            else:
                nc.gpsimd.collective_compute(
                    "AllReduce",
                    mybir.AluOpType.add,
                    replica_groups=[list(range(N_CORES))],
                    ins=[a_in.opt()],
                    outs=[a_out.opt()],
                )

        # ---- phase B: votes + uniform-route accumulation ----
        with (
            tc.tile_pool(name="pbtmp", bufs=1) as bpool,
            tc.tile_pool(name="wload", bufs=2) as wpool,
            tc.tile_pool(name="vpsum", bufs=4, space="PSUM") as vpsum,
            tc.tile_pool(name="vevict", bufs=2) as vepool,
        ):
            # stationaries ride the SWDGE ring so both HWDGE rings carry W
            s_sb = bpool.tile([128, NPAIR * 128], bf16, tag="s_sb")
            nc.gpsimd.dma_start(s_sb[:], s_d[:])
            acc4 = bpool.tile([128, 4 * OA], bf16, tag="acc4")
            for wg in range(NPAIR // 4):
                if wg == 2:
                    # dummy AllReduce: absorbs collective warmup and core
                    # skew while phase B keeps the engines busy
                    _all_reduce(warm_in, warm_out)
                # batched W load: 4 pairs per DMA (16 KB partition lines),
                # alternating the two HWDGE rings so W gets 2/3 of the DMA
                # round-robin against the spill ring
                wtg = wpool.tile([128, 4 * OA], bf16, tag="wt")
                weng = nc.sync if wg % 2 == 0 else nc.scalar
                weng.dma_start(
                    wtg[:], wp_d[:, 4 * wg * OA:4 * (wg + 1) * OA]
                )
                spill = 4 * wg >= R
                if spill:
                    veg = vepool.tile([128, 4 * OA], bf16, tag="ve")
                for t in range(4):
                    p = 4 * wg + t
                    wt = wtg[:, t * OA:(t + 1) * OA]
                    if spill:
                        ve_ap = veg[:, t * OA:(t + 1) * OA]
                    else:
                        ve_ap = votes_sb[:, p * OA:(p + 1) * OA]
                    ve_am = ve_ap.rearrange("p (a o) -> p a o", o=O)
                    j = p % 4
                    asl = acc4[:, j * OA:(j + 1) * OA]
                    asl_am = asl.rearrange("p (a o) -> p a o", o=O)
                    # half-pair PSUM tiles (depth-4 pipeline): the per-pair
                    # MM -> evict -> chain latency chain was the phase-B
                    # wall; four 2-bank buffers keep four halves in flight
                    for h in range(2):
                        vph = vpsum.tile([128, OA // 2], f32)
                        for q in range(2):
                            nc.tensor.matmul(
                                vph[:, q * 512:(q + 1) * 512],
                                s_sb[:, p * 128:(p + 1) * 128],
                                wt[:, h * 1024 + q * 512:
                                   h * 1024 + (q + 1) * 512],
                                start=True, stop=True,
                            )
                        # evict PSUM (o-major) -> bf16 a-major, one o-half
                        # at a time; mostly ScalarE, every 5th on Vector
                        vh_am = vph[:].rearrange("p (o a) -> p a o", a=A)
                        dst = ve_am[:, :, 32 * h:32 * (h + 1)]
                        if (2 * p + h) % 5 == 4:
                            nc.vector.tensor_copy(dst, vh_am)
                        else:
                            nc.scalar.copy(dst, vh_am)
                        # four interleaved bf16 accumulation chains
                        csl = asl_am[:, :, 32 * h:32 * (h + 1)]
                        src = ve_am[:, :, 32 * h:32 * (h + 1)]
                        if p < 4:
                            nc.vector.tensor_copy(csl, src)
                        else:
                            nc.vector.tensor_add(csl, csl, src)
                if spill:
                    # batched spill: 4 pairs per DMA on the SWDGE ring --
                    # spills are not on the AR1 critical path, so they get
                    # only 1/3 of the DMA round-robin
                    nc.gpsimd.dma_start(votes_dram[wg - R // 4][:], veg[:])
            # combine chains in f32, then parity halves; chunked so the
            # ship pipeline overlaps the phase-B tail
            accf = bpool.tile([128, OA], f32, tag="accf")
            tmpf = bpool.tile([128, OA], f32, tag="tmpf")
            acc_hi = ppool.tile([B, OA], f32, tag="acc_hi")
            partial0 = ppool.tile([B, OA], bf16, tag="partial")
            for cch in range(4):
                csl = slice(cch * 512, (cch + 1) * 512)
                nc.vector.tensor_add(
                    accf[:, csl], acc4[:, csl],
                    acc4[:, csl.start + OA:csl.stop + OA])
                nc.vector.tensor_add(
                    tmpf[:, csl], acc4[:, csl.start + 2 * OA:csl.stop + 2 * OA],
                    acc4[:, csl.start + 3 * OA:csl.stop + 3 * OA])
                nc.vector.tensor_add(accf[:, csl], accf[:, csl], tmpf[:, csl])
                nc.scalar.dma_start(acc_hi[:, csl], accf[B:128, csl])
                nc.vector.tensor_add(
                    partial0[:, csl], accf[0:B, csl], acc_hi[:, csl])
                nc.scalar.dma_start(ar_in[:, csl], partial0[:, csl])

        # ---- routing iterations ----
        act2 = ppool.tile([128, OA], bf16)   # a-major, bcast to both halves
        with (
            tc.tile_pool(name="vstream", bufs=3) as vspool,
            tc.tile_pool(name="prodp", bufs=1) as prpool,
            tc.tile_pool(name="passtmp", bufs=2) as tpool,
            tc.tile_pool(name="sqtmp", bufs=1) as qpool,
        ):
            s_full = qpool.tile([128, OA], f32, tag="s_full")
            pre = qpool.tile([128, OA], f32, tag="pre")
            nsq = qpool.tile([128, O], f32, tag="nsq")
            nrm = qpool.tile([128, O], f32, tag="nrm")
            den = qpool.tile([128, O], f32, tag="den")
            rec = qpool.tile([128, O], f32, tag="rec")
            fac = qpool.tile([128, O], f32, tag="fac")

            s_lo = qpool.tile([B, OA], bf16, tag="s_lo")
            for it in range(3):
                _all_reduce(ar_in, ar_out)

                # prefetch the first streamed votes groups during the
                # AllReduce window (they only depend on votes_dram)
                loaded = {}

                def _load(g):
                    if NGB * (g + 1) <= R:
                        return votes_sb[:, NGB * g * OA:NGB * (g + 1) * OA]
                    if g not in loaded:
                        vt = vspool.tile([128, NGB * OA], bf16, tag="vt")
                        nc.sync.dma_start(
                            vt[:], votes_dram[g - R // NGB][:])
                        loaded[g] = vt[:]
                    return loaded[g]

                if it < 2:
                    _load(R // NGB)
                    _load(R // NGB + 1)

                # consume on the low 64 partitions only (ACT ring: the SP
                # ring is busy with the votes prefetches)
                nc.scalar.dma_start(s_lo[:], ar_out[:])
                scale = LEAK_SCALE if it == 0 else 1.0
                # pre = s_lo * scale + bias
                nc.vector.scalar_tensor_tensor(
                    pre[0:B, :], s_lo[:], scale, bias_sb[0:B, :],
                    mybir.AluOpType.mult, mybir.AluOpType.add,
                )
                if it == 2:
                    # final tail: only ||act|| = nsq/(1+nsq) is needed
                    nc.vector.tensor_mul(s_full[0:B, :], pre[0:B, :],
                                         pre[0:B, :])
                    nc.vector.reduce_sum(
                        nsq[0:B, :],
                        s_full[0:B, :].rearrange("p (a o) -> p o a", o=O),
                        axis=mybir.AxisListType.X,
                    )
                    nc.vector.tensor_scalar_add(den[0:B, :], nsq[0:B, :], 1.0)
                    nc.vector.reciprocal(rec[0:B, :], den[0:B, :])
                    final = qpool.tile([128, O], f32, tag="final")
                    nc.vector.tensor_mul(final[0:B, :], nsq[0:B, :],
                                         rec[0:B, :])
                    nc.sync.dma_start(out_d[:], final[0:B, :])
                    break
                _squash_factors(
                    nc, pre[0:B, :], nsq[0:B, :], nrm[0:B, :], den[0:B, :],
                    rec[0:B, :], fac[0:B, :], s_full[0:B, :])
                # act2[p, (a, o)] = pre[p, (a, o)] * fac[p, o], low half,
                # then broadcast to the upper partition half
                nc.vector.tensor_tensor(
                    act2[0:B, :].rearrange("p (a o) -> p a o", o=O),
                    pre[0:B, :].rearrange("p (a o) -> p a o", o=O),
                    fac[0:B, :].unsqueeze(1).broadcast_to((B, A, O)),
                    mybir.AluOpType.mult,
                )
                nc.scalar.dma_start(act2[B:128, :], act2[0:B, :])

                # streaming pass over votes (a-major), NGB pairs per step,
                # software-pipelined one group deep: group g's cross-engine
                # softmax chain is emitted AROUND group g+1's big Vector
                # multiplies so Vector never head-of-line blocks
                with (
                    tc.tile_pool(name="pps", bufs=1, space="PSUM") as ppsum,
                    tc.tile_pool(name="dps", bufs=2, space="PSUM") as dpsum,
                ):
                    wps = ppsum.tile([B, OA], f32)
                    NG = NPAIR // NGB

                    def _agree(g):
                        """prod + fold + agreement matmuls."""
                        vt_ap = _load(g)
                        prod = prpool.tile([128, NGB * OA], bf16, tag="prod")
                        nc.vector.tensor_tensor(
                            prod[:].rearrange("p (t ao) -> p t ao", t=NGB),
                            vt_ap.rearrange("p (t ao) -> p t ao", t=NGB),
                            act2[:].unsqueeze(1).broadcast_to((128, NGB, OA)),
                            mybir.AluOpType.mult,
                        )
                        fv = prod[:].rearrange("p (t x) -> p t x", t=NGB)
                        nc.vector.tensor_add(
                            fv[:, :, 0:1024], fv[:, :, 0:1024],
                            fv[:, :, 1024:2048])
                        # finish the fold over a on the PE: 16 accumulating
                        # identity matmuls sum the remaining 16 a-slices
                        dps = dpsum.tile([128, NGB * O], f32, tag="dps")
                        for k in range(16):
                            nc.tensor.matmul(
                                dps[:], ident16[:],
                                fv[:, :, k * 64:(k + 1) * 64],
                                start=(k == 0), stop=(k == 15),
                            )
                        return (g, vt_ap, dps)

                    def _logexp(st):
                        """logits update + exp (ScalarE) for a prior group."""
                        g, vt_ap, dps = st
                        lp = logits[:, NGB * g * O:NGB * (g + 1) * O]
                        if it == 0:
                            nc.scalar.copy(lp, dps[:])
                        else:
                            nc.vector.tensor_add(lp, lp, dps[:])
                        expv = tpool.tile([128, NGB * O], f32, tag="expv")
                        nc.scalar.activation(
                            expv[:], lp, mybir.ActivationFunctionType.Exp)
                        return expv

                    def _route_wv(st, expv):
                        """softmax tail + route-weighted accumulate."""
                        g, vt_ap, dps = st
                        esum = tpool.tile([128, NGB], f32, tag="esum")
                        nc.vector.reduce_sum(
                            esum[:],
                            expv[:].rearrange("p (t o) -> p t o", t=NGB),
                            axis=mybir.AxisListType.X)
                        edn = tpool.tile([128, NGB], f32, tag="edn")
                        nc.vector.tensor_scalar_add(edn[:], esum[:], 1.0)
                        erc = tpool.tile([128, NGB], f32, tag="erc")
                        nc.vector.reciprocal(erc[:], edn[:])
                        route = tpool.tile([128, NGB * O], bf16, tag="route")
                        nc.vector.tensor_tensor(
                            route[:].rearrange("p (t o) -> p t o", t=NGB),
                            expv[:].rearrange("p (t o) -> p t o", t=NGB),
                            erc[:].unsqueeze(-1).broadcast_to((128, NGB, O)),
                            mybir.AluOpType.mult,
                        )
                        # wv = vt * route (broadcast over outer atom axis)
                        wv = tpool.tile([128, NGB * OA], bf16, tag="wv")
                        nc.vector.tensor_tensor(
                            wv[:].rearrange(
                                "p (t a o) -> p t a o", t=NGB, o=O),
                            vt_ap.rearrange(
                                "p (t a o) -> p t a o", t=NGB, o=O),
                            route[:].rearrange(
                                "p (t o) -> p t o", t=NGB).unsqueeze(
                                2).broadcast_to((128, NGB, A, O)),
                            mybir.AluOpType.mult,
                        )
                        # accumulate on PE with the parity-fold stationary:
                        # wps[64, :] += fold64.T @ wv folds the two capsule
                        # parity halves during the accumulate, so the
                        # AllReduce partial ships with no extra hops
                        for q in range(4 * NGB):
                            nc.tensor.matmul(
                                wps[:, (q % 4) * 512:(q % 4) * 512 + 512],
                                fold64[:],
                                wv[:, q * 512:(q + 1) * 512],
                                start=(g == 0 and q < 4),
                                stop=(g == NG - 1 and q >= 4 * NGB - 4),
                            )

                    pend = None
                    for g in range(NG):
                        if g + 1 < NG:
                            _load(g + 1)   # one group of DMA lead
                        pexp = _logexp(pend) if pend is not None else None
                        st = _agree(g)
                        if pend is not None:
                            _route_wv(pend, pexp)
                        pend = st
                    _route_wv(pend, _logexp(pend))
                    # evict + ship in 512-wide chunks (parity already
                    # folded by the PE stationary)
                    partial = ppool.tile([B, OA], bf16, tag="partial")
                    for cch in range(4):
                        csl = slice(cch * 512, (cch + 1) * 512)
                        nc.scalar.copy(partial[:, csl], wps[:, csl])
                        nc.scalar.dma_start(ar_in[:, csl], partial[:, csl])


def make_in_maps(inputs, W, biases):
    """Host-side prep: per-core block-diag stationaries, paired W, a-major
    biases, all bf16/f32 as the device expects."""
    inputs = np.asarray(inputs, dtype=np.float32)
    W = np.asarray(W, dtype=np.float32)
    biases = np.asarray(biases, dtype=np.float32)
    bias_am = np.ascontiguousarray(biases.T).reshape(OA)  # a-major
    in_maps = []
    for k in range(N_CORES):
        sl = slice(k * IC, (k + 1) * IC)
        x = inputs[:, sl, :]                        # [B, IC, C]
        xT = np.ascontiguousarray(x.transpose(1, 2, 0))  # [IC, C, B]
        S = np.zeros((NPAIR, 128, 128), dtype=np.float32)
        S[:, 0:64, 0:64] = xT[0::2]
        S[:, 64:128, 64:128] = xT[1::2]
        # device wants S as [128, NPAIR*128]: [k_row, (pair, col)]
        S = np.ascontiguousarray(S.transpose(1, 0, 2)).reshape(128, NPAIR * 128)
        Wp = W[sl].reshape(NPAIR, 128, OA)          # [pair, (2 caps x C), OA]
        # partition-major: [k_row, (pair, oa)] so group loads are contiguous
        Wp = np.ascontiguousarray(Wp.transpose(1, 0, 2)).reshape(128, NPAIR * OA)
        in_maps.append({
            "S": S.astype(bf16_np),
            "Wp": Wp.astype(bf16_np),
            "biases": bias_am,
        })
    return in_maps


_NC_CACHE = {}


def kernel(inputs, W, biases):
    if "nc" not in _NC_CACHE:
        _NC_CACHE["nc"] = _build()
    nc = _NC_CACHE["nc"]
    in_maps = make_in_maps(inputs, W, biases)
    res = run_bass_kernel_spmd(nc, in_maps, core_ids=list(range(N_CORES)))
    return res.results[0]["out"]


if __name__ == "__main__":
    rng = np.random.default_rng(0)
    inputs = rng.standard_normal((B, I_FULL, C)).astype(np.float32)
    W = (rng.standard_normal((I_FULL, C, OA)) * 0.02).astype(np.float32)
    biases = (rng.standard_normal((O, A)) * 0.01).astype(np.float32)
    out = kernel(inputs, W, biases)
    print("out shape:", out.shape, "sample:", out[0, :4])



# revision 36
# speedup vs baseline: 1.1699x; 1.1699x over previous
"""DigitCaps routing kernel for 8 Trainium2 NeuronCores.

Sharding: input_dim (1024 primary capsules) split 8 ways; per-core
preactivation partial sums are AllReduced each routing iteration.

Per core (I_c = 128 local capsules = 64 pairs):
  phase B: votes_pair = S_p.T @ Wp_p with S_p a host-built block-diagonal
           [128,128] stationary (inputsT of the 2 capsules on the diagonal),
           so one moving stream of W computes both capsules' votes.
           Uniform-route preactivation (iteration 0) accumulates on DVE in
           four interleaved bf16 chains, combined in f32.
           A tiny dummy AllReduce early in phase B absorbs collective
           warmup + core skew so the real AllReduces run at floor latency.
  pass k (k=1,2): stream votes (SBUF-resident for the first R pairs, DRAM
           for the rest), compute agreement delta -> logits -> leaky softmax
           route -> route-weighted partial preactivation (PE identity-
           accumulate).
  Each iteration: AllReduce [64, 2048] partials, squash locally.

All per-(b,o,a) tensors are kept a-major (a outer, o inner) so the
agreement fold over atoms is a contiguous halving chain.
"""

import sys

if '/opt/trn_rl_repo' not in sys.path:
    sys.path.insert(0, '/opt/trn_rl_repo')

import numpy as np
import ml_dtypes

import concourse.bacc as bacc
import concourse.mybir as mybir
import concourse.tile as tile
from concourse import masks
from concourse.bass_utils import run_bass_kernel_spmd

N_CORES = 8
B = 64          # batch
I_FULL = 1024   # primary capsules
C = 64          # input atoms
O = 64          # output capsules
A = 32          # output atoms
OA = O * A      # 2048
IC = I_FULL // N_CORES   # 128 local capsules
NPAIR = IC // 2          # 64
NGB = 4                  # pairs per streaming group
R = 8                    # SBUF-resident pairs (must be multiple of NGB)

f32 = mybir.dt.float32
bf16 = mybir.dt.bfloat16
f8 = mybir.dt.float8e4
bf16_np = ml_dtypes.bfloat16
f8_np = ml_dtypes.float8_e4m3

W_SCALE = 64.0  # host pre-scale so fp8 W stays in normal range

LEAK_SCALE = 1.0 / (O + 1)  # route0 value: softmax of 65 zero logits


def _squash_factors(nc, pre, nsq, nrm, den, rec, fac, sq):
    """pre [P, OA] f32 a-major -> fac [P, O] f32 (act = pre * fac)."""
    nc.vector.tensor_mul(sq, pre, pre)
    nc.vector.reduce_sum(
        nsq, sq.rearrange("p (a o) -> p o a", o=O),
        axis=mybir.AxisListType.X,
    )
    nc.scalar.sqrt(nrm, nsq)
    nc.scalar.add(den, nsq, 1.0)
    nc.vector.reciprocal(rec, den)
    nc.vector.tensor_mul(fac, nrm, rec)


def _build(sim_mode=False):
    nc = bacc.Bacc("TRN2", target_bir_lowering=False, debug=False,
                   num_devices=1 if sim_mode else N_CORES)
    with tile.TileContext(nc) as tc:
        _emit(nc, tc, sim_mode)
    nc.compile()
    return nc


def _emit(nc, tc, sim_mode=False):
    s_d = nc.dram_tensor("S", [128, NPAIR * 128], bf16, kind="ExternalInput")
    # W partition-major: per partition one contiguous (pair, oa) run, so a
    # 4-pair group load is 128 contiguous 16 KB descriptors (near line rate)
    wp_d = nc.dram_tensor("Wp", [128, NPAIR * OA], bf16, kind="ExternalInput")
    b_d = nc.dram_tensor("biases", [OA], f32, kind="ExternalInput")
    out_d = nc.dram_tensor("out", [B, O], f32, kind="ExternalOutput")

    with (
        tc.tile_pool(name="const", bufs=1) as cpool,
        tc.tile_pool(name="persist", bufs=1) as ppool,
        tc.tile_pool(name="dram", bufs=1, space="DRAM") as dpool,
    ):
        ident16 = cpool.tile([128, 128], bf16)
        masks.make_identity(nc, ident16[:])
        # parity-fold stationary: 1 at (k, k%64) -- folds the two capsule
        # parity halves during the PE preactivation accumulate
        fold64 = cpool.tile([128, 64], bf16)
        nc.vector.tensor_add(fold64[:], ident16[:, 0:64], ident16[:, 64:128])
        bias_sb = cpool.tile([128, OA], f32)  # a-major broadcast
        nc.scalar.dma_start(
            bias_sb[:], b_d[:].unsqueeze(0).broadcast_to((128, OA))
        )

        votes_sb = ppool.tile([128, R * OA], bf16)
        logits = ppool.tile([128, NPAIR * O], bf16)  # [(par,b), (pair, o)]
        votes_dram = dpool.tile([(NPAIR - R) // NGB, 128, NGB * OA], bf16)
        ar_in = dpool.tile([B, OA], bf16)
        ar_out = dpool.tile([B, OA], bf16)
        warm_in = dpool.tile([B, OA], bf16)
        warm_out = dpool.tile([B, OA], bf16)

        def _all_reduce(a_in, a_out):
            if sim_mode:
                nc.sync.dma_start(a_out[:], a_in[:])
            else:
                nc.gpsimd.collective_compute(
                    "AllReduce",
                    mybir.AluOpType.add,
                    replica_groups=[list(range(N_CORES))],
                    ins=[a_in.opt()],
                    outs=[a_out.opt()],
                )

        # ---- phase B: votes + uniform-route accumulation ----
        with (
            tc.tile_pool(name="pbtmp", bufs=1) as bpool,
            tc.tile_pool(name="wload", bufs=4) as wpool,
            tc.tile_pool(name="vpsum", bufs=2, space="PSUM") as vpsum,
            tc.tile_pool(name="vevict", bufs=2) as vepool,
        ):
            # stationaries ride the SWDGE ring so both HWDGE rings carry W
            s_sb = bpool.tile([128, NPAIR * 128], bf16, tag="s_sb")
            nc.gpsimd.dma_start(s_sb[:], s_d[:])
            acc4 = bpool.tile([128, 4 * OA], bf16, tag="acc4")
            for wg in range(NPAIR // 2):
                if wg == 4:
                    # dummy AllReduce: absorbs collective warmup and core
                    # skew while phase B keeps the engines busy
                    _all_reduce(warm_in, warm_out)
                # batched W load: 2 pairs per DMA, 4 buffers deep so W
                # issuance decouples from the evict/chain cadence;
                # alternating the two HWDGE rings
                wtg = wpool.tile([128, 2 * OA], bf16, tag="wt")
                weng = nc.sync if wg % 2 == 0 else nc.scalar
                weng.dma_start(
                    wtg[:], wp_d[:, 2 * wg * OA:2 * (wg + 1) * OA]
                )
                spill = 2 * wg >= R
                if spill and wg % 2 == 0:
                    veg = vepool.tile([128, 4 * OA], bf16, tag="ve")
                for t in range(2):
                    p = 2 * wg + t
                    wt = wtg[:, t * OA:(t + 1) * OA]
                    vps = vpsum.tile([128, OA], f32)
                    for q in range(4):
                        nc.tensor.matmul(
                            vps[:, q * 512:(q + 1) * 512],
                            s_sb[:, p * 128:(p + 1) * 128],
                            wt[:, q * 512:(q + 1) * 512],
                            start=True, stop=True,
                        )
                    # evict PSUM (o-major) -> bf16 a-major; split across
                    # ScalarE (most pairs) and Vector (every 6th) so the
                    # 1x-rate PSUM eviction is not a single-engine wall
                    if spill:
                        ve_ap = veg[:, (p % 4) * OA:(p % 4 + 1) * OA]
                    else:
                        ve_ap = votes_sb[:, p * OA:(p + 1) * OA]
                    if p % 6 == 5:
                        nc.vector.tensor_copy(
                            ve_ap.rearrange("p (a o) -> p a o", o=O),
                            vps[:].rearrange("p (o a) -> p a o", a=A),
                        )
                    else:
                        nc.scalar.copy(
                            ve_ap.rearrange("p (a o) -> p a o", o=O),
                            vps[:].rearrange("p (o a) -> p a o", a=A),
                        )
                    # four interleaved bf16 accumulation chains
                    j = p % 4
                    asl = acc4[:, j * OA:(j + 1) * OA]
                    if p < 4:
                        nc.vector.tensor_copy(asl, ve_ap)
                    else:
                        nc.vector.tensor_add(asl, asl, ve_ap)
                if spill and wg % 2 == 1:
                    # batched spill: 4 pairs per DMA on the SWDGE ring --
                    # spills are not on the AR1 critical path
                    nc.gpsimd.dma_start(votes_dram[wg // 2 - R // 4][:], veg[:])
            # combine chains in f32, then parity halves; chunked so the
            # ship pipeline overlaps the phase-B tail
            accf = bpool.tile([128, OA], f32, tag="accf")
            tmpf = bpool.tile([128, OA], f32, tag="tmpf")
            acc_hi = ppool.tile([B, OA], f32, tag="acc_hi")
            partial0 = ppool.tile([B, OA], bf16, tag="partial")
            for cch in range(4):
                csl = slice(cch * 512, (cch + 1) * 512)
                nc.vector.tensor_add(
                    accf[:, csl], acc4[:, csl],
                    acc4[:, csl.start + OA:csl.stop + OA])
                nc.vector.tensor_add(
                    tmpf[:, csl], acc4[:, csl.start + 2 * OA:csl.stop + 2 * OA],
                    acc4[:, csl.start + 3 * OA:csl.stop + 3 * OA])
                nc.vector.tensor_add(accf[:, csl], accf[:, csl], tmpf[:, csl])
                nc.scalar.dma_start(acc_hi[:, csl], accf[B:128, csl])
                nc.vector.tensor_add(
                    partial0[:, csl], accf[0:B, csl], acc_hi[:, csl])
                nc.scalar.dma_start(ar_in[:, csl], partial0[:, csl])

        # ---- routing iterations ----
        act2 = ppool.tile([128, OA], bf16)   # a-major, bcast to both halves
        with (
            tc.tile_pool(name="vstream", bufs=3) as vspool,
            tc.tile_pool(name="prodp", bufs=1) as prpool,
            tc.tile_pool(name="passtmp", bufs=2) as tpool,
            tc.tile_pool(name="sqtmp", bufs=1) as qpool,
        ):
            s_full = qpool.tile([128, OA], f32, tag="s_full")
            pre = qpool.tile([128, OA], f32, tag="pre")
            nsq = qpool.tile([128, O], f32, tag="nsq")
            nrm = qpool.tile([128, O], f32, tag="nrm")
            den = qpool.tile([128, O], f32, tag="den")
            rec = qpool.tile([128, O], f32, tag="rec")
            fac = qpool.tile([128, O], f32, tag="fac")

            s_lo = qpool.tile([B, OA], bf16, tag="s_lo")
            for it in range(3):
                _all_reduce(ar_in, ar_out)

                # prefetch the first streamed votes groups during the
                # AllReduce window (they only depend on votes_dram)
                loaded = {}

                def _load(g):
                    if NGB * (g + 1) <= R:
                        return votes_sb[:, NGB * g * OA:NGB * (g + 1) * OA]
                    if g not in loaded:
                        vt = vspool.tile([128, NGB * OA], bf16, tag="vt")
                        nc.sync.dma_start(
                            vt[:], votes_dram[g - R // NGB][:])
                        loaded[g] = vt[:]
                    return loaded[g]

                if it < 2:
                    _load(R // NGB)
                    _load(R // NGB + 1)

                # consume on the low 64 partitions only (ACT ring: the SP
                # ring is busy with the votes prefetches)
                nc.scalar.dma_start(s_lo[:], ar_out[:])
                scale = LEAK_SCALE if it == 0 else 1.0
                # pre = s_lo * scale + bias
                nc.vector.scalar_tensor_tensor(
                    pre[0:B, :], s_lo[:], scale, bias_sb[0:B, :],
                    mybir.AluOpType.mult, mybir.AluOpType.add,
                )
                if it == 2:
                    # final tail: only ||act|| = nsq/(1+nsq) is needed
                    nc.vector.tensor_mul(s_full[0:B, :], pre[0:B, :],
                                         pre[0:B, :])
                    nc.vector.reduce_sum(
                        nsq[0:B, :],
                        s_full[0:B, :].rearrange("p (a o) -> p o a", o=O),
                        axis=mybir.AxisListType.X,
                    )
                    nc.vector.tensor_scalar_add(den[0:B, :], nsq[0:B, :], 1.0)
                    nc.vector.reciprocal(rec[0:B, :], den[0:B, :])
                    final = qpool.tile([128, O], f32, tag="final")
                    nc.vector.tensor_mul(final[0:B, :], nsq[0:B, :],
                                         rec[0:B, :])
                    nc.sync.dma_start(out_d[:], final[0:B, :])
                    break
                _squash_factors(
                    nc, pre[0:B, :], nsq[0:B, :], nrm[0:B, :], den[0:B, :],
                    rec[0:B, :], fac[0:B, :], s_full[0:B, :])
                # act2[p, (a, o)] = pre[p, (a, o)] * fac[p, o], low half,
                # then broadcast to the upper partition half
                nc.vector.tensor_tensor(
                    act2[0:B, :].rearrange("p (a o) -> p a o", o=O),
                    pre[0:B, :].rearrange("p (a o) -> p a o", o=O),
                    fac[0:B, :].unsqueeze(1).broadcast_to((B, A, O)),
                    mybir.AluOpType.mult,
                )
                nc.scalar.dma_start(act2[B:128, :], act2[0:B, :])

                # streaming pass over votes (a-major), NGB pairs per step,
                # software-pipelined one group deep: group g's cross-engine
                # softmax chain is emitted AROUND group g+1's big Vector
                # multiplies so Vector never head-of-line blocks
                with (
                    tc.tile_pool(name="pps", bufs=1, space="PSUM") as ppsum,
                    tc.tile_pool(name="dps", bufs=2, space="PSUM") as dpsum,
                ):
                    wps = ppsum.tile([B, OA], f32)
                    NG = NPAIR // NGB

                    def _agree(g):
                        """prod + fold + agreement matmuls."""
                        vt_ap = _load(g)
                        prod = prpool.tile([128, NGB * OA], bf16, tag="prod")
                        nc.vector.tensor_tensor(
                            prod[:].rearrange("p (t ao) -> p t ao", t=NGB),
                            vt_ap.rearrange("p (t ao) -> p t ao", t=NGB),
                            act2[:].unsqueeze(1).broadcast_to((128, NGB, OA)),
                            mybir.AluOpType.mult,
                        )
                        fv = prod[:].rearrange("p (t x) -> p t x", t=NGB)
                        nc.vector.tensor_add(
                            fv[:, :, 0:1024], fv[:, :, 0:1024],
                            fv[:, :, 1024:2048])
                        # finish the fold over a on the PE: 16 accumulating
                        # identity matmuls sum the remaining 16 a-slices
                        dps = dpsum.tile([128, NGB * O], f32, tag="dps")
                        for k in range(16):
                            nc.tensor.matmul(
                                dps[:], ident16[:],
                                fv[:, :, k * 64:(k + 1) * 64],
                                start=(k == 0), stop=(k == 15),
                            )
                        return (g, vt_ap, dps)

                    def _logexp(st):
                        """logits update + exp (ScalarE) for a prior group."""
                        g, vt_ap, dps = st
                        lp = logits[:, NGB * g * O:NGB * (g + 1) * O]
                        if it == 0:
                            nc.scalar.copy(lp, dps[:])
                        else:
                            nc.vector.tensor_add(lp, lp, dps[:])
                        expv = tpool.tile([128, NGB * O], f32, tag="expv")
                        nc.scalar.activation(
                            expv[:], lp, mybir.ActivationFunctionType.Exp)
                        return expv

                    def _route_wv(st, expv):
                        """softmax tail + route-weighted accumulate."""
                        g, vt_ap, dps = st
                        esum = tpool.tile([128, NGB], f32, tag="esum")
                        nc.vector.reduce_sum(
                            esum[:],
                            expv[:].rearrange("p (t o) -> p t o", t=NGB),
                            axis=mybir.AxisListType.X)
                        edn = tpool.tile([128, NGB], f32, tag="edn")
                        nc.vector.tensor_scalar_add(edn[:], esum[:], 1.0)
                        erc = tpool.tile([128, NGB], f32, tag="erc")
                        nc.vector.reciprocal(erc[:], edn[:])
                        route = tpool.tile([128, NGB * O], bf16, tag="route")
                        nc.vector.tensor_tensor(
                            route[:].rearrange("p (t o) -> p t o", t=NGB),
                            expv[:].rearrange("p (t o) -> p t o", t=NGB),
                            erc[:].unsqueeze(-1).broadcast_to((128, NGB, O)),
                            mybir.AluOpType.mult,
                        )
                        # wv = vt * route (broadcast over outer atom axis)
                        wv = tpool.tile([128, NGB * OA], bf16, tag="wv")
                        nc.vector.tensor_tensor(
                            wv[:].rearrange(
                                "p (t a o) -> p t a o", t=NGB, o=O),
                            vt_ap.rearrange(
                                "p (t a o) -> p t a o", t=NGB, o=O),
                            route[:].rearrange(
                                "p (t o) -> p t o", t=NGB).unsqueeze(
                                2).broadcast_to((128, NGB, A, O)),
                            mybir.AluOpType.mult,
                        )
                        # accumulate on PE with the parity-fold stationary:
                        # wps[64, :] += fold64.T @ wv folds the two capsule
                        # parity halves during the accumulate, so the
                        # AllReduce partial ships with no extra hops
                        for q in range(4 * NGB):
                            nc.tensor.matmul(
                                wps[:, (q % 4) * 512:(q % 4) * 512 + 512],
                                fold64[:],
                                wv[:, q * 512:(q + 1) * 512],
                                start=(g == 0 and q < 4),
                                stop=(g == NG - 1 and q >= 4 * NGB - 4),
                            )

                    pend = None
                    for g in range(NG):
                        if g + 1 < NG:
                            _load(g + 1)   # one group of DMA lead
                        pexp = _logexp(pend) if pend is not None else None
                        st = _agree(g)
                        if pend is not None:
                            _route_wv(pend, pexp)
                        pend = st
                    _route_wv(pend, _logexp(pend))
                    # evict + ship in 512-wide chunks (parity already
                    # folded by the PE stationary)
                    partial = ppool.tile([B, OA], bf16, tag="partial")
                    for cch in range(4):
                        csl = slice(cch * 512, (cch + 1) * 512)
                        nc.scalar.copy(partial[:, csl], wps[:, csl])
                        nc.scalar.dma_start(ar_in[:, csl], partial[:, csl])


def make_in_maps(inputs, W, biases):
    """Host-side prep: per-core block-diag stationaries, paired W, a-major
    biases, all bf16/f32 as the device expects."""
    inputs = np.asarray(inputs, dtype=np.float32)
    W = np.asarray(W, dtype=np.float32)
    biases = np.asarray(biases, dtype=np.float32)
    bias_am = np.ascontiguousarray(biases.T).reshape(OA)  # a-major
    in_maps = []
    for k in range(N_CORES):
        sl = slice(k * IC, (k + 1) * IC)
        x = inputs[:, sl, :]                        # [B, IC, C]
        xT = np.ascontiguousarray(x.transpose(1, 2, 0))  # [IC, C, B]
        S = np.zeros((NPAIR, 128, 128), dtype=np.float32)
        S[:, 0:64, 0:64] = xT[0::2]
        S[:, 64:128, 64:128] = xT[1::2]
        # device wants S as [128, NPAIR*128]: [k_row, (pair, col)]
        S = np.ascontiguousarray(S.transpose(1, 0, 2)).reshape(128, NPAIR * 128)
        Wp = W[sl].reshape(NPAIR, 128, OA)          # [pair, (2 caps x C), OA]
        # partition-major: [k_row, (pair, oa)] so group loads are contiguous
        Wp = np.ascontiguousarray(Wp.transpose(1, 0, 2)).reshape(128, NPAIR * OA)
        in_maps.append({
            "S": S.astype(bf16_np),
            "Wp": Wp.astype(bf16_np),
            "biases": bias_am,
        })
    return in_maps


_NC_CACHE = {}


def kernel(inputs, W, biases):
    if "nc" not in _NC_CACHE:
        _NC_CACHE["nc"] = _build()
    nc = _NC_CACHE["nc"]
    in_maps = make_in_maps(inputs, W, biases)
    res = run_bass_kernel_spmd(nc, in_maps, core_ids=list(range(N_CORES)))
    return res.results[0]["out"]


if __name__ == "__main__":
    rng = np.random.default_rng(0)
    inputs = rng.standard_normal((B, I_FULL, C)).astype(np.float32)
    W = (rng.standard_normal((I_FULL, C, OA)) * 0.02).astype(np.float32)
    biases = (rng.standard_normal((O, A)) * 0.01).astype(np.float32)
    out = kernel(inputs, W, biases)
    print("out shape:", out.shape, "sample:", out[0, :4])



# revision 43
# speedup vs baseline: 1.1954x; 1.0218x over previous
"""DigitCaps routing kernel for 8 Trainium2 NeuronCores.

Sharding: input_dim (1024 primary capsules) split 8 ways; per-core
preactivation partial sums are AllReduced each routing iteration.

Per core (I_c = 128 local capsules = 64 pairs):
  phase B: votes_pair = S_p.T @ Wp_p with S_p a host-built block-diagonal
           [128,128] stationary (inputsT of the 2 capsules on the diagonal),
           so one moving stream of W computes both capsules' votes.
           Uniform-route preactivation (iteration 0) accumulates on DVE in
           four interleaved bf16 chains, combined in f32.
           A tiny dummy AllReduce early in phase B absorbs collective
           warmup + core skew so the real AllReduces run at floor latency.
  pass k (k=1,2): stream votes (SBUF-resident for the first R pairs, DRAM
           for the rest), compute agreement delta -> logits -> leaky softmax
           route -> route-weighted partial preactivation (PE identity-
           accumulate).
  Each iteration: AllReduce [64, 2048] partials, squash locally.

All per-(b,o,a) tensors are kept a-major (a outer, o inner) so the
agreement fold over atoms is a contiguous halving chain.
"""

import sys

if '/opt/trn_rl_repo' not in sys.path:
    sys.path.insert(0, '/opt/trn_rl_repo')

import numpy as np
import ml_dtypes

import concourse.bacc as bacc
import concourse.mybir as mybir
import concourse.tile as tile
from concourse import masks
from concourse.bass_utils import run_bass_kernel_spmd

N_CORES = 8
B = 64          # batch
I_FULL = 1024   # primary capsules
C = 64          # input atoms
O = 64          # output capsules
A = 32          # output atoms
OA = O * A      # 2048
IC = I_FULL // N_CORES   # 128 local capsules
NPAIR = IC // 2          # 64
NGB = 4                  # pairs per streaming group
R = 8                    # SBUF-resident pairs (must be multiple of NGB)

f32 = mybir.dt.float32
bf16 = mybir.dt.bfloat16
f8 = mybir.dt.float8e4
bf16_np = ml_dtypes.bfloat16
f8_np = ml_dtypes.float8_e4m3

W_SCALE = 64.0  # host pre-scale so fp8 W stays in normal range

LEAK_SCALE = 1.0 / (O + 1)  # route0 value: softmax of 65 zero logits


def _squash_factors(nc, pre, nsq, nrm, den, rec, fac, sq):
    """pre [P, OA] f32 a-major -> fac [P, O] f32 (act = pre * fac)."""
    nc.vector.tensor_mul(sq, pre, pre)
    nc.vector.reduce_sum(
        nsq, sq.rearrange("p (a o) -> p o a", o=O),
        axis=mybir.AxisListType.X,
    )
    nc.scalar.sqrt(nrm, nsq)
    nc.scalar.add(den, nsq, 1.0)
    nc.vector.reciprocal(rec, den)
    nc.vector.tensor_mul(fac, nrm, rec)


def _build(sim_mode=False):
    nc = bacc.Bacc("TRN2", target_bir_lowering=False, debug=False,
                   num_devices=1 if sim_mode else N_CORES)
    with tile.TileContext(nc) as tc:
        _emit(nc, tc, sim_mode)
    nc.compile()
    return nc


def _emit(nc, tc, sim_mode=False):
    s_d = nc.dram_tensor("S", [128, NPAIR * 128], bf16, kind="ExternalInput")
    # W partition-major: per partition one contiguous (pair, oa) run, so a
    # 4-pair group load is 128 contiguous 16 KB descriptors (near line rate)
    wp_d = nc.dram_tensor("Wp", [128, NPAIR * OA], bf16, kind="ExternalInput")
    b_d = nc.dram_tensor("biases", [OA], f32, kind="ExternalInput")
    out_d = nc.dram_tensor("out", [B, O], f32, kind="ExternalOutput")

    with (
        tc.tile_pool(name="const", bufs=1) as cpool,
        tc.tile_pool(name="persist", bufs=1) as ppool,
        tc.tile_pool(name="dram", bufs=1, space="DRAM") as dpool,
    ):
        ident16 = cpool.tile([128, 128], bf16)
        masks.make_identity(nc, ident16[:])
        # parity-fold stationary: 1 at (k, k%64) -- folds the two capsule
        # parity halves during the PE preactivation accumulate
        fold64 = cpool.tile([128, 64], bf16)
        nc.vector.tensor_add(fold64[:], ident16[:, 0:64], ident16[:, 64:128])
        bias_sb = cpool.tile([128, OA], f32)  # a-major broadcast
        nc.scalar.dma_start(
            bias_sb[:], b_d[:].unsqueeze(0).broadcast_to((128, OA))
        )

        votes_sb = ppool.tile([128, R * OA], bf16)
        logits = ppool.tile([128, NPAIR * O], bf16)  # [(par,b), (pair, o)]
        votes_dram = dpool.tile([NPAIR - R, 128, OA], bf16)
        ar_in = dpool.tile([B, OA], bf16)
        ar_out = dpool.tile([B, OA], bf16)
        warm_in = dpool.tile([B, OA], bf16)
        warm_out = dpool.tile([B, OA], bf16)

        def _all_reduce(a_in, a_out):
            if sim_mode:
                nc.sync.dma_start(a_out[:], a_in[:])
            else:
                nc.gpsimd.collective_compute(
                    "AllReduce",
                    mybir.AluOpType.add,
                    replica_groups=[list(range(N_CORES))],
                    ins=[a_in.opt()],
                    outs=[a_out.opt()],
                )

        # ---- phase B: votes + uniform-route accumulation ----
        with (
            tc.tile_pool(name="pbtmp", bufs=1) as bpool,
            tc.tile_pool(name="wload", bufs=4) as wpool,
            tc.tile_pool(name="vpsum", bufs=2, space="PSUM") as vpsum,
            tc.tile_pool(name="vevict", bufs=4) as vepool,
        ):
            # stationaries ride the SWDGE ring so both HWDGE rings carry W
            s_sb = bpool.tile([128, NPAIR * 128], bf16, tag="s_sb")
            nc.gpsimd.dma_start(s_sb[:], s_d[:])
            acc4 = bpool.tile([128, 4 * OA], bf16, tag="acc4")
            for wg in range(NPAIR // 2):
                if wg == 4:
                    # dummy AllReduce: absorbs collective warmup and core
                    # skew while phase B keeps the engines busy
                    _all_reduce(warm_in, warm_out)
                # batched W load: 2 pairs per DMA, 4 buffers deep so W
                # issuance decouples from the evict/chain cadence;
                # alternating the two HWDGE rings
                wtg = wpool.tile([128, 2 * OA], bf16, tag="wt")
                weng = nc.sync if wg % 2 == 0 else nc.scalar
                weng.dma_start(
                    wtg[:], wp_d[:, 2 * wg * OA:2 * (wg + 1) * OA]
                )
                spill = 2 * wg >= R
                if spill:
                    veg = vepool.tile([128, 2 * OA], bf16, tag="ve")
                for t in range(2):
                    p = 2 * wg + t
                    wt = wtg[:, t * OA:(t + 1) * OA]
                    vps = vpsum.tile([128, OA], f32)
                    for q in range(4):
                        nc.tensor.matmul(
                            vps[:, q * 512:(q + 1) * 512],
                            s_sb[:, p * 128:(p + 1) * 128],
                            wt[:, q * 512:(q + 1) * 512],
                            start=True, stop=True,
                        )
                    # evict PSUM (o-major) -> bf16 a-major; split across
                    # ScalarE (most pairs) and Vector (every 6th) so the
                    # 1x-rate PSUM eviction is not a single-engine wall
                    if spill:
                        ve_ap = veg[:, t * OA:(t + 1) * OA]
                    else:
                        ve_ap = votes_sb[:, p * OA:(p + 1) * OA]
                    if p % 6 == 5:
                        nc.vector.tensor_copy(
                            ve_ap.rearrange("p (a o) -> p a o", o=O),
                            vps[:].rearrange("p (o a) -> p a o", a=A),
                        )
                    else:
                        nc.scalar.copy(
                            ve_ap.rearrange("p (a o) -> p a o", o=O),
                            vps[:].rearrange("p (o a) -> p a o", a=A),
                        )
                    # four interleaved bf16 accumulation chains
                    j = p % 4
                    asl = acc4[:, j * OA:(j + 1) * OA]
                    if p < 4:
                        nc.vector.tensor_copy(asl, ve_ap)
                    else:
                        nc.vector.tensor_add(asl, asl, ve_ap)
                if spill:
                    # 2-pair spill chunks, 4 buffers deep, on the SWDGE
                    # ring: spill completion no longer paces the pipeline
                    nc.gpsimd.dma_start(
                        votes_dram[2 * wg - R:2 * wg - R + 2].rearrange(
                            "c p x -> p c x"),
                        veg[:].rearrange("p (c x) -> p c x", c=2))
            # combine chains in f32, then parity halves; chunked so the
            # ship pipeline overlaps the phase-B tail
            accf = bpool.tile([128, OA], f32, tag="accf")
            tmpf = bpool.tile([128, OA], f32, tag="tmpf")
            acc_hi = ppool.tile([B, OA], f32, tag="acc_hi")
            partial0 = ppool.tile([B, OA], bf16, tag="partial")
            for cch in range(4):
                csl = slice(cch * 512, (cch + 1) * 512)
                nc.vector.tensor_add(
                    accf[:, csl], acc4[:, csl],
                    acc4[:, csl.start + OA:csl.stop + OA])
                nc.vector.tensor_add(
                    tmpf[:, csl], acc4[:, csl.start + 2 * OA:csl.stop + 2 * OA],
                    acc4[:, csl.start + 3 * OA:csl.stop + 3 * OA])
                nc.vector.tensor_add(accf[:, csl], accf[:, csl], tmpf[:, csl])
                nc.scalar.dma_start(acc_hi[:, csl], accf[B:128, csl])
                nc.vector.tensor_add(
                    partial0[:, csl], accf[0:B, csl], acc_hi[:, csl])
                nc.scalar.dma_start(ar_in[:, csl], partial0[:, csl])

        # ---- routing iterations ----
        act2 = ppool.tile([128, OA], bf16)   # a-major, bcast to both halves
        with (
            tc.tile_pool(name="vstream", bufs=3) as vspool,
            tc.tile_pool(name="prodp", bufs=1) as prpool,
            tc.tile_pool(name="passtmp", bufs=2) as tpool,
            tc.tile_pool(name="sqtmp", bufs=1) as qpool,
        ):
            s_full = qpool.tile([128, OA], f32, tag="s_full")
            pre = qpool.tile([128, OA], f32, tag="pre")
            nsq = qpool.tile([128, O], f32, tag="nsq")
            nrm = qpool.tile([128, O], f32, tag="nrm")
            den = qpool.tile([128, O], f32, tag="den")
            rec = qpool.tile([128, O], f32, tag="rec")
            fac = qpool.tile([128, O], f32, tag="fac")

            s_lo = qpool.tile([B, OA], bf16, tag="s_lo")
            for it in range(3):
                _all_reduce(ar_in, ar_out)

                # prefetch the first streamed votes groups during the
                # AllReduce window (they only depend on votes_dram)
                loaded = {}

                def _load(g):
                    if NGB * (g + 1) <= R:
                        return votes_sb[:, NGB * g * OA:NGB * (g + 1) * OA]
                    if g not in loaded:
                        vt = vspool.tile([128, NGB * OA], bf16, tag="vt")
                        base = NGB * g - R
                        nc.sync.dma_start(
                            vt[:].rearrange("p (c x) -> p c x", c=NGB),
                            votes_dram[base:base + NGB].rearrange(
                                "c p x -> p c x"))
                        loaded[g] = vt[:]
                    return loaded[g]

                if it < 2:
                    _load(R // NGB)
                    _load(R // NGB + 1)

                # consume on the low 64 partitions only (ACT ring: the SP
                # ring is busy with the votes prefetches)
                nc.scalar.dma_start(s_lo[:], ar_out[:])
                scale = LEAK_SCALE if it == 0 else 1.0
                # pre = s_lo * scale + bias
                nc.vector.scalar_tensor_tensor(
                    pre[0:B, :], s_lo[:], scale, bias_sb[0:B, :],
                    mybir.AluOpType.mult, mybir.AluOpType.add,
                )
                if it == 2:
                    # final tail: only ||act|| = nsq/(1+nsq) is needed
                    nc.vector.tensor_mul(s_full[0:B, :], pre[0:B, :],
                                         pre[0:B, :])
                    nc.vector.reduce_sum(
                        nsq[0:B, :],
                        s_full[0:B, :].rearrange("p (a o) -> p o a", o=O),
                        axis=mybir.AxisListType.X,
                    )
                    nc.vector.tensor_scalar_add(den[0:B, :], nsq[0:B, :], 1.0)
                    nc.vector.reciprocal(rec[0:B, :], den[0:B, :])
                    final = qpool.tile([128, O], f32, tag="final")
                    nc.vector.tensor_mul(final[0:B, :], nsq[0:B, :],
                                         rec[0:B, :])
                    nc.sync.dma_start(out_d[:], final[0:B, :])
                    break
                _squash_factors(
                    nc, pre[0:B, :], nsq[0:B, :], nrm[0:B, :], den[0:B, :],
                    rec[0:B, :], fac[0:B, :], s_full[0:B, :])
                # act2[p, (a, o)] = pre[p, (a, o)] * fac[p, o], low half,
                # then broadcast to the upper partition half
                nc.vector.tensor_tensor(
                    act2[0:B, :].rearrange("p (a o) -> p a o", o=O),
                    pre[0:B, :].rearrange("p (a o) -> p a o", o=O),
                    fac[0:B, :].unsqueeze(1).broadcast_to((B, A, O)),
                    mybir.AluOpType.mult,
                )
                nc.scalar.dma_start(act2[B:128, :], act2[0:B, :])

                # streaming pass over votes (a-major), NGB pairs per step,
                # software-pipelined one group deep: group g's cross-engine
                # softmax chain is emitted AROUND group g+1's big Vector
                # multiplies so Vector never head-of-line blocks
                with (
                    tc.tile_pool(name="pps", bufs=1, space="PSUM") as ppsum,
                    tc.tile_pool(name="dps", bufs=2, space="PSUM") as dpsum,
                ):
                    wps = ppsum.tile([B, OA], f32)
                    NG = NPAIR // NGB

                    def _agree(g):
                        """prod + fold + agreement matmuls."""
                        vt_ap = _load(g)
                        prod = prpool.tile([128, NGB * OA], bf16, tag="prod")
                        nc.vector.tensor_tensor(
                            prod[:].rearrange("p (t ao) -> p t ao", t=NGB),
                            vt_ap.rearrange("p (t ao) -> p t ao", t=NGB),
                            act2[:].unsqueeze(1).broadcast_to((128, NGB, OA)),
                            mybir.AluOpType.mult,
                        )
                        fv = prod[:].rearrange("p (t x) -> p t x", t=NGB)
                        nc.vector.tensor_add(
                            fv[:, :, 0:1024], fv[:, :, 0:1024],
                            fv[:, :, 1024:2048])
                        # finish the fold over a on the PE: 16 accumulating
                        # identity matmuls sum the remaining 16 a-slices
                        dps = dpsum.tile([128, NGB * O], f32, tag="dps")
                        for k in range(16):
                            nc.tensor.matmul(
                                dps[:], ident16[:],
                                fv[:, :, k * 64:(k + 1) * 64],
                                start=(k == 0), stop=(k == 15),
                            )
                        return (g, vt_ap, dps)

                    def _logexp(st):
                        """logits update + exp (ScalarE) for a prior group."""
                        g, vt_ap, dps = st
                        lp = logits[:, NGB * g * O:NGB * (g + 1) * O]
                        if it == 0:
                            nc.scalar.copy(lp, dps[:])
                        else:
                            nc.vector.tensor_add(lp, lp, dps[:])
                        expv = tpool.tile([128, NGB * O], f32, tag="expv")
                        nc.scalar.activation(
                            expv[:], lp, mybir.ActivationFunctionType.Exp)
                        return expv

                    def _route_wv(st, expv):
                        """softmax tail + route-weighted accumulate."""
                        g, vt_ap, dps = st
                        esum = tpool.tile([128, NGB], f32, tag="esum")
                        nc.vector.reduce_sum(
                            esum[:],
                            expv[:].rearrange("p (t o) -> p t o", t=NGB),
                            axis=mybir.AxisListType.X)
                        edn = tpool.tile([128, NGB], f32, tag="edn")
                        nc.vector.tensor_scalar_add(edn[:], esum[:], 1.0)
                        erc = tpool.tile([128, NGB], f32, tag="erc")
                        nc.vector.reciprocal(erc[:], edn[:])
                        route = tpool.tile([128, NGB * O], bf16, tag="route")
                        nc.vector.tensor_tensor(
                            route[:].rearrange("p (t o) -> p t o", t=NGB),
                            expv[:].rearrange("p (t o) -> p t o", t=NGB),
                            erc[:].unsqueeze(-1).broadcast_to((128, NGB, O)),
                            mybir.AluOpType.mult,
                        )
                        # wv = vt * route (broadcast over outer atom axis)
                        wv = tpool.tile([128, NGB * OA], bf16, tag="wv")
                        nc.vector.tensor_tensor(
                            wv[:].rearrange(
                                "p (t a o) -> p t a o", t=NGB, o=O),
                            vt_ap.rearrange(
                                "p (t a o) -> p t a o", t=NGB, o=O),
                            route[:].rearrange(
                                "p (t o) -> p t o", t=NGB).unsqueeze(
                                2).broadcast_to((128, NGB, A, O)),
                            mybir.AluOpType.mult,
                        )
                        # accumulate on PE with the parity-fold stationary:
                        # wps[64, :] += fold64.T @ wv folds the two capsule
                        # parity halves during the accumulate, so the
                        # AllReduce partial ships with no extra hops
                        for q in range(4 * NGB):
                            nc.tensor.matmul(
                                wps[:, (q % 4) * 512:(q % 4) * 512 + 512],
                                fold64[:],
                                wv[:, q * 512:(q + 1) * 512],
                                start=(g == 0 and q < 4),
                                stop=(g == NG - 1 and q >= 4 * NGB - 4),
                            )

                    pend = None
                    for g in range(NG):
                        if g + 1 < NG:
                            _load(g + 1)   # one group of DMA lead
                        pexp = _logexp(pend) if pend is not None else None
                        st = _agree(g)
                        if pend is not None:
                            _route_wv(pend, pexp)
                        pend = st
                    _route_wv(pend, _logexp(pend))
                    # evict + ship in 512-wide chunks (parity already
                    # folded by the PE stationary)
                    partial = ppool.tile([B, OA], bf16, tag="partial")
                    for cch in range(4):
                        csl = slice(cch * 512, (cch + 1) * 512)
                        nc.scalar.copy(partial[:, csl], wps[:, csl])
                        nc.scalar.dma_start(ar_in[:, csl], partial[:, csl])


def make_in_maps(inputs, W, biases):
    """Host-side prep: per-core block-diag stationaries, paired W, a-major
    biases, all bf16/f32 as the device expects."""
    inputs = np.asarray(inputs, dtype=np.float32)
    W = np.asarray(W, dtype=np.float32)
    biases = np.asarray(biases, dtype=np.float32)
    bias_am = np.ascontiguousarray(biases.T).reshape(OA)  # a-major
    in_maps = []
    for k in range(N_CORES):
        sl = slice(k * IC, (k + 1) * IC)
        x = inputs[:, sl, :]                        # [B, IC, C]
        xT = np.ascontiguousarray(x.transpose(1, 2, 0))  # [IC, C, B]
        S = np.zeros((NPAIR, 128, 128), dtype=np.float32)
        S[:, 0:64, 0:64] = xT[0::2]
        S[:, 64:128, 64:128] = xT[1::2]
        # device wants S as [128, NPAIR*128]: [k_row, (pair, col)]
        S = np.ascontiguousarray(S.transpose(1, 0, 2)).reshape(128, NPAIR * 128)
        Wp = W[sl].reshape(NPAIR, 128, OA)          # [pair, (2 caps x C), OA]
        # partition-major: [k_row, (pair, oa)] so group loads are contiguous
        Wp = np.ascontiguousarray(Wp.transpose(1, 0, 2)).reshape(128, NPAIR * OA)
        in_maps.append({
            "S": S.astype(bf16_np),
            "Wp": Wp.astype(bf16_np),
            "biases": bias_am,
        })
    return in_maps


_NC_CACHE = {}


def kernel(inputs, W, biases):
    if "nc" not in _NC_CACHE:
        _NC_CACHE["nc"] = _build()
    nc = _NC_CACHE["nc"]
    in_maps = make_in_maps(inputs, W, biases)
    res = run_bass_kernel_spmd(nc, in_maps, core_ids=list(range(N_CORES)))
    return res.results[0]["out"]


if __name__ == "__main__":
    rng = np.random.default_rng(0)
    inputs = rng.standard_normal((B, I_FULL, C)).astype(np.float32)
    W = (rng.standard_normal((I_FULL, C, OA)) * 0.02).astype(np.float32)
    biases = (rng.standard_normal((O, A)) * 0.01).astype(np.float32)
    out = kernel(inputs, W, biases)
    print("out shape:", out.shape, "sample:", out[0, :4])

